# revision 24
# baseline (speedup 1.0000x reference)
"""Trainium2 Bass kernel for CrossModalAttention.

Reference computation (B=1, C=64, N=8192 voxels):
  two cross-attention directions (CT queries over MRI keys/values and vice
  versa), each with an 8192x8192 attention matrix, fused output projection.

Sharding: each of the 8 cores owns 1024 query voxels for BOTH directions,
computes K/V over the full sequence locally (features are only 2 MB per
modality), and produces its own (64, 1024) slice of the output through the
final projection. No collectives; the host concatenates the 8 slices.

Device algorithm ("transposed world", no transposes of large tensors):
  The K projection is folded into the query side (associativity:
  (Wk f)^T q = f^T (Wk^T q)), so scores read the fp16 features directly:
    scores^T (j,i) = matmul(lhsT=feat_aug[:, j-chunk] (65,128),
                            rhs=q''_d (65,512)),  q''_d = [Wk^T q_d; bk.q_d]
  exp on ScalarE straight out of PSUM (max-subtraction skipped: |s| <= ~1.2),
  batched 3 PSUM banks per ACTIVATE to amortize the 352-cycle overhead.
  AV is flipped to out=(i,c) so the PE output partition dim is the full 128:
    att^T[i, 0:65] += matmul(lhsT=exp chunk (j,128i), rhs=V^T_aug (j,65))
  V^T_aug = feat_aug^T @ Wv' where Wv' carries the bias row and a final
  [0..0,1] column, so column 64 of att^T accumulates the softmax
  denominator for free. All four 128-query chains of one 512-query block
  share a single PSUM bank: only the first matmul uses start=True (clears
  the bank); later chains' first writes land on has_written=0 cells and
  overwrite, which initializes them correctly.
  Normalize = per-partition reciprocal of column 64 + tensor_scalar mult,
  then a 128x64 PE transpose per subblock rebuilds the channel-major
  fused tile for the fp32 final projection.

Precision: matmul operands are fp16 (PE streams 1 col/cycle with fast
weight load; fp32 is 4x slower, float32r 2x, and bf16's 8-bit mantissa
loses 10x accuracy for identical speed -- all values here are far inside
fp16 range). Accumulation is always fp32 in PSUM; softmax denominator,
normalization, transposes and the final projection are fp32. Rounding
errors of q/k/exp/v average out over the 8192-key softmax: end-to-end
error ~2e-4.
"""

from contextlib import ExitStack

import numpy as np

import concourse.bass as bass
import concourse.mybir as mybir
import concourse.tile as tile
from concourse import bacc
from concourse.bass_utils import run_bass_kernel_spmd

F32 = mybir.dt.float32
F16 = mybir.dt.float16
C = 64          # channels
N = 8192        # voxels (8*32*32)
NCORES = 8
NQ = N // NCORES      # 1024 queries per core
IH = 512              # query block (PSUM bank width in f32)
NIH = NQ // IH        # 2
NSUB = IH // 128      # 4 query subblocks per block (AV lhsT width)
JCH = 128             # key chunk (AV contraction tile)
NJ = N // JCH         # 64
GRP = 3               # score banks per exp group (3 banks -> one wide ACT op)
VGW = 4               # vT chunks per projection group (4*65 f32 fits one bank)
W = C + 1             # 65: augmented channel dim
VCOLS = NJ * W        # vT storage: 64 chunks x 65 cols (65th col = denom ones)
NFS = 8               # feature DMA subtiles
FSW = N // NFS        # 1024 cols per subtile
JPS = FSW // JCH      # 8 j-chunks per feature subtile


def _emit_feat_load(nc, featp, feat_dram, tag, name):
    subs = []
    for s in range(NFS):
        t = featp.tile([W, FSW], F16, tag=tag, name=f"{name}{s}")
        nc.sync.dma_start(t[:], feat_dram[:, FSW * s : FSW * (s + 1)])
        subs.append(t)
    return subs


def _emit_q_proj(nc, pools, wq_sb, qsrc, wcol, name):
    """q_d (64, NQ) = Wq_aug^T @ qsrc_aug (bias via the features' ones row)."""
    qp, sp = pools["qp"], pools["sp"]
    q = qp.tile([C, NQ], F16, tag="q", name=name)
    for h in range(NIH):
        pq = sp.tile([C, IH], F32, tag="ps", name=f"pq_{name}{h}")
        nc.tensor.matmul(
            pq[:],
            lhsT=wq_sb[:, wcol : wcol + C],
            rhs=qsrc[:, IH * h : IH * (h + 1)],
            start=True,
            stop=True,
        )
        nc.vector.tensor_copy(q[:, IH * h : IH * (h + 1)], pq[:])
    return q


def _emit_qq_proj(nc, pools, wkb_sb, q, wcol, name):
    """q''_d (65, NQ) = [Wk | bk]^T @ q_d -- the K projection folded into Q."""
    qp, sp = pools["qp"], pools["sp"]
    qq = qp.tile([W, NQ], F16, tag="qq", name=name)
    for h in range(NIH):
        pq = sp.tile([W, IH], F32, tag="ps", name=f"pqq_{name}{h}")
        nc.tensor.matmul(
            pq[:],
            lhsT=wkb_sb[:, wcol : wcol + W],
            rhs=q[:, IH * h : IH * (h + 1)],
            start=True,
            stop=True,
        )
        nc.vector.tensor_copy(qq[:, IH * h : IH * (h + 1)], pq[:])
    return qq


def _emit_v_proj(nc, pools, wv_sb, fs, wcol, name):
    """vT_aug (128j x 65) chunks = feat_aug^T @ Wv' (ones column built in)."""
    vp, sp = pools["vp"], pools["sp"]
    vT = vp.tile([JCH, VCOLS], F16, tag="v", name=name)
    for g in range(NJ // VGW):
        pv = sp.tile([JCH, VGW * W], F32, tag="ps", name=f"pv_{name}{g}")
        for cc in range(VGW):
            j = VGW * g + cc
            nc.tensor.matmul(
                pv[:, W * cc : W * (cc + 1)],
                lhsT=fs[j // JPS][:, JCH * (j % JPS) : JCH * (j % JPS + 1)],
                rhs=wv_sb[:, wcol : wcol + W],
                start=True,
                stop=True,
            )
        nc.vector.tensor_copy(vT[:, W * VGW * g : W * VGW * (g + 1)], pv[:])
    return vT


def _emit_attention(nc, pools, fa, qq, vT, fused_t, d):
    sp, pap, mp, ep, npl = (
        pools["sp"], pools["pap"], pools["mp"], pools["ep"], pools["np"],
    )
    identity = pools["identity"]
    for ih in range(NIH):
        # one bank holds all four (128, 65) accumulation chains
        pacc = pap.tile([JCH, NSUB * W], F32, tag="pacc", name=f"pacc{d}{ih}")
        for jg in range((NJ + GRP - 1) // GRP):
            js = list(range(GRP * jg, min(GRP * (jg + 1), NJ)))
            ps = sp.tile([JCH, GRP * IH], F32, tag="ps", name=f"ps{d}{ih}{jg}")
            for idx, j in enumerate(js):
                nc.tensor.matmul(
                    ps[:, IH * idx : IH * (idx + 1)],
                    lhsT=fa[j // JPS][:, JCH * (j % JPS) : JCH * (j % JPS + 1)],
                    rhs=qq[:, IH * ih : IH * (ih + 1)],
                    start=True,
                    stop=True,
                )
            et = ep.tile([JCH, GRP * IH], F16, tag="exp", name=f"et{d}{ih}{jg}")
            nc.scalar.activation(
                et[:, : IH * len(js)],
                ps[:, : IH * len(js)],
                mybir.ActivationFunctionType.Exp,
            )
            for idx, j in enumerate(js):
                for isub in range(NSUB):
                    nc.tensor.matmul(
                        pacc[:, W * isub : W * (isub + 1)],
                        lhsT=et[:, IH * idx + JCH * isub : IH * idx + JCH * (isub + 1)],
                        rhs=vT[:, W * j : W * (j + 1)],
                        start=(j == 0 and isub == 0),
                        stop=(j == NJ - 1 and isub == NSUB - 1),
                        skip_group_check=True,
                    )
        # normalize per query (partition): r = 1 / denom-column
        r4 = npl.tile([JCH, NSUB], F32, tag="r4", name=f"r4{d}{ih}")
        nc.vector.reciprocal(
            r4[:].rearrange("p (i w) -> p i w", w=1),
            pacc[:].rearrange("p (i w) -> p i w", w=W)[:, :, C : C + 1],
        )
        attT = npl.tile([JCH, NSUB * C], F32, tag="attT", name=f"attT{d}{ih}")
        for isub in range(NSUB):
            nc.vector.tensor_scalar_mul(
                attT[:, C * isub : C * (isub + 1)],
                pacc[:, W * isub : W * isub + C],
                r4[:, isub : isub + 1],
            )
        # transpose each (128, 64) subblock back to channel-major
        pt = mp.tile([C, IH], F32, tag="mp", name=f"pt{d}{ih}")
        for isub in range(NSUB):
            nc.tensor.transpose(
                pt[:, JCH * isub : JCH * (isub + 1)],
                attT[:, C * isub : C * (isub + 1)],
                identity[:],
            )
        nc.vector.tensor_copy(fused_t[ih][C * d : C * (d + 1), :], pt[:])


def _build_program(
    ctx, tc, ct, mri, qsrc_ct, qsrc_mri, wq, wkb, wv, woT, bo, ident, out
):
    nc = tc.nc
    wpool = ctx.enter_context(tc.tile_pool(name="wpool", bufs=1))
    featp = ctx.enter_context(tc.tile_pool(name="feat", bufs=2 * NFS))
    pools = {
        "qp": ctx.enter_context(tc.tile_pool(name="qp", bufs=2)),
        "vp": ctx.enter_context(tc.tile_pool(name="vp", bufs=2)),
        "ep": ctx.enter_context(tc.tile_pool(name="ep", bufs=4)),
        "np": ctx.enter_context(tc.tile_pool(name="npool", bufs=2)),
        "sp": ctx.enter_context(
            tc.tile_pool(name="spsum", bufs=2, space="PSUM")
        ),
        "pap": ctx.enter_context(
            tc.tile_pool(name="paccp", bufs=1, space="PSUM")
        ),
        "mp": ctx.enter_context(tc.tile_pool(name="mpsum", bufs=1, space="PSUM")),
    }
    fp = ctx.enter_context(tc.tile_pool(name="fusedp", bufs=2))
    op = ctx.enter_context(tc.tile_pool(name="outp", bufs=2))

    wq_sb = wpool.tile([W, 2 * C], F16, name="wq_sb")
    nc.sync.dma_start(wq_sb[:], wq[:])
    wkb_sb = wpool.tile([C, 2 * W], F16, name="wkb_sb")
    nc.sync.dma_start(wkb_sb[:], wkb[:])
    wv_sb = wpool.tile([W, 2 * W], F16, name="wv_sb")
    nc.sync.dma_start(wv_sb[:], wv[:])
    woT_sb = wpool.tile([2 * C, C], F32, name="woT_sb")
    nc.sync.dma_start(woT_sb[:], woT[:])
    bo_sb = wpool.tile([C, 1], F32, name="bo_sb")
    nc.sync.dma_start(bo_sb[:], bo[:])
    ident_sb = wpool.tile([JCH, JCH], F32, name="ident_sb")
    nc.sync.dma_start(ident_sb[:], ident[:])
    pools["identity"] = ident_sb

    fused_t = [
        fp.tile([2 * C, IH], F32, tag="fused", name=f"fused{ih}")
        for ih in range(NIH)
    ]

    # tiny query-source DMAs go first so they don't queue behind the 2 MB
    # of feature DMAs (HWDGE queues are FIFO)
    qsc = pools["qp"].tile([W, NQ], F16, tag="qsrc", name="qsc")
    nc.sync.dma_start(qsc[:], qsrc_ct[:])
    qsm = pools["qp"].tile([W, NQ], F16, tag="qsrc", name="qsm")
    nc.sync.dma_start(qsm[:], qsrc_mri[:])

    # mri side first: it feeds direction 0 (CT queries over MRI K/V)
    fs_mri = _emit_feat_load(nc, featp, mri, "fsm", "fmri")
    fs_ct = _emit_feat_load(nc, featp, ct, "fsc", "fct")
    q_ct = _emit_q_proj(nc, pools, wq_sb, qsc, 0 * C, "q_ct")
    qq_d0 = _emit_qq_proj(nc, pools, wkb_sb, q_ct, 0 * W, "qq_d0")
    vT_mri = _emit_v_proj(nc, pools, wv_sb, fs_mri, 0 * W, "vT_mri")
    q_mri = _emit_q_proj(nc, pools, wq_sb, qsm, 1 * C, "q_mri")

    # direction 0 while CT projections stream in behind it
    _emit_attention(nc, pools, fs_mri, qq_d0, vT_mri, fused_t, 0)

    qq_d1 = _emit_qq_proj(nc, pools, wkb_sb, q_mri, 1 * W, "qq_d1")
    vT_ct = _emit_v_proj(nc, pools, wv_sb, fs_ct, 1 * W, "vT_ct")

    _emit_attention(nc, pools, fs_ct, qq_d1, vT_ct, fused_t, 1)

    for ih in range(NIH):
        po = pools["mp"].tile([C, IH], F32, tag="mp", name=f"po{ih}")
        nc.tensor.matmul(
            po[:], lhsT=woT_sb[:], rhs=fused_t[ih][:], start=True, stop=True
        )
        ot = op.tile([C, IH], F32, tag="ot", name=f"ot{ih}")
        nc.vector.tensor_scalar_add(ot[:], po[:], bo_sb[:])
        nc.sync.dma_start(out[:, IH * ih : IH * (ih + 1)], ot[:])


def build_bass():
    nc = bacc.Bacc("TRN2", target_bir_lowering=False, debug=False)
    ct = nc.dram_tensor("ct_feat", [W, N], F16, kind="ExternalInput").ap()
    mri = nc.dram_tensor("mri_feat", [W, N], F16, kind="ExternalInput").ap()
    qsrc_ct = nc.dram_tensor("qsrc_ct", [W, NQ], F16, kind="ExternalInput").ap()
    qsrc_mri = nc.dram_tensor("qsrc_mri", [W, NQ], F16, kind="ExternalInput").ap()
    wq = nc.dram_tensor("wq", [W, 2 * C], F16, kind="ExternalInput").ap()
    wkb = nc.dram_tensor("wkb", [C, 2 * W], F16, kind="ExternalInput").ap()
    wv = nc.dram_tensor("wv", [W, 2 * W], F16, kind="ExternalInput").ap()
    woT = nc.dram_tensor("woT", [2 * C, C], F32, kind="ExternalInput").ap()
    bo = nc.dram_tensor("bo", [C, 1], F32, kind="ExternalInput").ap()
    ident = nc.dram_tensor("ident", [JCH, JCH], F32, kind="ExternalInput").ap()
    out = nc.dram_tensor("out", [C, NQ], F32, kind="ExternalOutput").ap()

    with tile.TileContext(nc) as tc, ExitStack() as ctx:
        _build_program(
            ctx, tc, ct, mri, qsrc_ct, qsrc_mri, wq, wkb, wv, woT, bo, ident, out
        )
    nc.compile()
    return nc


def _aug(w, b):
    # (out,in) weight + (out,) bias -> lhsT-ready [w.T; b] of shape (in+1, out)
    return np.concatenate(
        [np.asarray(w, np.float32).T, np.asarray(b, np.float32)[None, :]], axis=0
    )


def _wv_pack(w, b):
    # (65, 65): [[wv.T; bv] | e_last]: extra column accumulates the denominator
    m = np.zeros((W, W), np.float32)
    m[:, :C] = _aug(w, b)
    m[C, C] = 1.0
    return m


def _wkb_pack(w, b):
    # (64, 65): [wk | bk] -- K projection folded into the query side
    return np.concatenate(
        [np.asarray(w, np.float32), np.asarray(b, np.float32)[:, None]], axis=1
    )


def prepare_inputs(inputs):
    scale = np.float32(1.0 / np.sqrt(C))
    ct = np.asarray(inputs["ct_features"], np.float32).reshape(C, N)
    mri = np.asarray(inputs["mri_features"], np.float32).reshape(C, N)
    ones = np.ones((1, N), np.float32)
    ct_aug = np.concatenate([ct, ones], axis=0).astype(np.float16)
    mri_aug = np.concatenate([mri, ones], axis=0).astype(np.float16)
    wq = np.concatenate(
        [
            _aug(np.asarray(inputs["wq_ct"]) * scale, np.asarray(inputs["bq_ct"]) * scale),
            _aug(np.asarray(inputs["wq_mri"]) * scale, np.asarray(inputs["bq_mri"]) * scale),
        ],
        axis=1,
    ).astype(np.float16)
    wkb = np.concatenate(
        [_wkb_pack(inputs["wk_mri"], inputs["bk_mri"]),
         _wkb_pack(inputs["wk_ct"], inputs["bk_ct"])],
        axis=1,
    ).astype(np.float16)
    wv = np.concatenate(
        [_wv_pack(inputs["wv_mri"], inputs["bv_mri"]),
         _wv_pack(inputs["wv_ct"], inputs["bv_ct"])],
        axis=1,
    ).astype(np.float16)
    woT = np.ascontiguousarray(np.asarray(inputs["wo"], np.float32).T)
    bo = np.ascontiguousarray(np.asarray(inputs["bo"], np.float32)[:, None])
    ident = np.eye(JCH, dtype=np.float32)

    in_maps = []
    for i in range(NCORES):
        sl = slice(NQ * i, NQ * (i + 1))
        in_maps.append(
            {
                "ct_feat": ct_aug,
                "mri_feat": mri_aug,
                "qsrc_ct": np.ascontiguousarray(ct_aug[:, sl]),
                "qsrc_mri": np.ascontiguousarray(mri_aug[:, sl]),
                "wq": wq,
                "wkb": wkb,
                "wv": wv,
                "woT": woT,
                "bo": bo,
                "ident": ident,
            }
        )
    return in_maps


def assemble_output(results):
    out = np.concatenate([results[i]["out"] for i in range(NCORES)], axis=1)
    return out.reshape(1, C, 8, 32, 32)


_NC_CACHE = None


def _get_nc():
    global _NC_CACHE
    if _NC_CACHE is None:
        _NC_CACHE = build_bass()
    return _NC_CACHE


def kernel(**inputs):
    nc = _get_nc()
    in_maps = prepare_inputs(inputs)
    res = run_bass_kernel_spmd(nc, in_maps, list(range(NCORES)))
    return assemble_output(res.results)


if __name__ == "__main__":
    nc = build_bass()
    print("built OK")


# revision 32
# speedup vs baseline: 1.0523x; 1.0523x over previous
"""Trainium2 Bass kernel for CrossModalAttention.

Reference computation (B=1, C=64, N=8192 voxels):
  two cross-attention directions (CT queries over MRI keys/values and vice
  versa), each with an 8192x8192 attention matrix, fused output projection.

Sharding: each of the 8 cores owns 1024 query voxels for BOTH directions,
computes K/V over the full sequence locally (features are only 2 MB per
modality), and produces its own (64, 1024) slice of the output through the
final projection. No collectives; the host concatenates the 8 slices.

Device algorithm ("transposed world", no transposes of large tensors):
  The K projection is folded into the query side (associativity:
  (Wk f)^T q = f^T (Wk^T q)), so scores read the fp16 features directly:
    scores^T (j,i) = matmul(lhsT=feat_aug[:, j-chunk] (65,128),
                            rhs=q''_d (65,512)),  q''_d = [Wk^T q_d; bk.q_d]
  exp on ScalarE straight out of PSUM (max-subtraction skipped: |s| <= ~1.2),
  batched 3 PSUM banks per ACTIVATE to amortize the 352-cycle overhead.
  AV is flipped to out=(i,c) so the PE output partition dim is the full 128:
    att^T[i, 0:65] += matmul(lhsT=exp chunk (j,128i), rhs=V^T_aug (j,65))
  V^T_aug = feat_aug^T @ Wv' where Wv' carries the bias row and a final
  [0..0,1] column, so column 64 of att^T accumulates the softmax
  denominator for free. All four 128-query chains of one 512-query block
  share a single PSUM bank: only the first matmul uses start=True (clears
  the bank); later chains' first writes land on has_written=0 cells and
  overwrite, which initializes them correctly.
  Normalize = per-partition reciprocal of column 64 + tensor_scalar mult,
  then a 128x64 PE transpose per subblock rebuilds the channel-major
  fused tile for the fp32 final projection.

Precision: matmul operands are fp16 (PE streams 1 col/cycle with fast
weight load; fp32 is 4x slower, float32r 2x, and bf16's 8-bit mantissa
loses 10x accuracy for identical speed -- all values here are far inside
fp16 range). Accumulation is always fp32 in PSUM; softmax denominator,
normalization, transposes and the final projection are fp32. Rounding
errors of q/k/exp/v average out over the 8192-key softmax: end-to-end
error ~2e-4.
"""

from contextlib import ExitStack

import numpy as np

import concourse.bass as bass
import concourse.mybir as mybir
import concourse.tile as tile
from concourse import bacc
from concourse.bass_utils import run_bass_kernel_spmd

F32 = mybir.dt.float32
F16 = mybir.dt.float16
C = 64          # channels
N = 8192        # voxels (8*32*32)
NCORES = 8
NQ = N // NCORES      # 1024 queries per core
IH = 512              # query block (PSUM bank width in f32)
NIH = NQ // IH        # 2
NSUB = IH // 128      # 4 query subblocks per block (AV lhsT width)
JCH = 128             # key chunk (AV contraction tile)
NJ = N // JCH         # 64
GRP = 3               # score banks per exp group (3 banks -> one wide ACT op)
VGW = 4               # vT chunks per projection group (4*65 f32 fits one bank)
W = C + 1             # 65: augmented channel dim
VCOLS = NJ * W        # vT storage: 64 chunks x 65 cols (65th col = denom ones)
NFS = 8               # feature DMA subtiles
FSW = N // NFS        # 1024 cols per subtile
JPS = FSW // JCH      # 8 j-chunks per feature subtile


def _emit_feat_load(nc, featp, feat_dram, tag, name):
    subs = []
    for s in range(NFS):
        t = featp.tile([W, FSW], F16, tag=tag, name=f"{name}{s}")
        nc.sync.dma_start(t[:], feat_dram[:, FSW * s : FSW * (s + 1)])
        subs.append(t)
    return subs


def _emit_qq_proj(nc, pools, wqq_sb, qsrc, wcol, name):
    """q''_d (65, NQ) = (Wq_aug @ [Wk|bk])^T @ qsrc_aug -- the Q projection and
    the K projection (folded onto the query side) composed on the host."""
    qp, sp = pools["qp"], pools["sp"]
    qq = qp.tile([W, NQ], F16, tag="qq", name=name)
    for h in range(NIH):
        pq = sp.tile([W, IH], F32, tag="ps", name=f"pqq_{name}{h}")
        nc.tensor.matmul(
            pq[:],
            lhsT=wqq_sb[:, wcol : wcol + W],
            rhs=qsrc[:, IH * h : IH * (h + 1)],
            start=True,
            stop=True,
        )
        nc.vector.tensor_copy(qq[:, IH * h : IH * (h + 1)], pq[:])
    return qq


def _emit_v_proj(nc, pools, wv_sb, fs, wcol, name):
    """vT_aug (128j x 65) chunks = feat_aug^T @ Wv' (ones column built in)."""
    vp, sp = pools["vp"], pools["sp"]
    vT = vp.tile([JCH, VCOLS], F16, tag="v", name=name)
    for g in range(NJ // VGW):
        pv = sp.tile([JCH, VGW * W], F32, tag="ps", name=f"pv_{name}{g}")
        for cc in range(VGW):
            j = VGW * g + cc
            nc.tensor.matmul(
                pv[:, W * cc : W * (cc + 1)],
                lhsT=fs[j // JPS][:, JCH * (j % JPS) : JCH * (j % JPS + 1)],
                rhs=wv_sb[:, wcol : wcol + W],
                start=True,
                stop=True,
            )
        nc.vector.tensor_copy(vT[:, W * VGW * g : W * VGW * (g + 1)], pv[:])
    return vT


def _emit_attention(nc, pools, fa, qq, vT, fused_t, d):
    sp, pap, mp, ep, npl = (
        pools["sp"], pools["pap"], pools["mp"], pools["ep"], pools["np"],
    )
    identity = pools["identity"]
    for ih in range(NIH):
        # one bank holds all four (128, 65) accumulation chains
        pacc = pap.tile([JCH, NSUB * W], F32, tag="pacc", name=f"pacc{d}{ih}")
        for jg in range((NJ + GRP - 1) // GRP):
            js = list(range(GRP * jg, min(GRP * (jg + 1), NJ)))
            ps = sp.tile([JCH, GRP * IH], F32, tag="ps", name=f"ps{d}{ih}{jg}")
            for idx, j in enumerate(js):
                nc.tensor.matmul(
                    ps[:, IH * idx : IH * (idx + 1)],
                    lhsT=fa[j // JPS][:, JCH * (j % JPS) : JCH * (j % JPS + 1)],
                    rhs=qq[:, IH * ih : IH * (ih + 1)],
                    start=True,
                    stop=True,
                )
            et = ep.tile([JCH, GRP * IH], F16, tag="exp", name=f"et{d}{ih}{jg}")
            nc.scalar.activation(
                et[:, : IH * len(js)],
                ps[:, : IH * len(js)],
                mybir.ActivationFunctionType.Exp,
            )
            for idx, j in enumerate(js):
                for isub in range(NSUB):
                    nc.tensor.matmul(
                        pacc[:, W * isub : W * (isub + 1)],
                        lhsT=et[:, IH * idx + JCH * isub : IH * idx + JCH * (isub + 1)],
                        rhs=vT[:, W * j : W * (j + 1)],
                        start=(j == 0 and isub == 0),
                        stop=(j == NJ - 1 and isub == NSUB - 1),
                        skip_group_check=True,
                    )
        # normalize per query (partition): r = 1 / denom-column
        r4 = npl.tile([JCH, NSUB], F32, tag="r4", name=f"r4{d}{ih}")
        nc.vector.reciprocal(
            r4[:].rearrange("p (i w) -> p i w", w=1),
            pacc[:].rearrange("p (i w) -> p i w", w=W)[:, :, C : C + 1],
        )
        attT = npl.tile([JCH, NSUB * C], F32, tag="attT", name=f"attT{d}{ih}")
        for isub in range(NSUB):
            nc.vector.tensor_scalar_mul(
                attT[:, C * isub : C * (isub + 1)],
                pacc[:, W * isub : W * isub + C],
                r4[:, isub : isub + 1],
            )
        # transpose each (128, 64) subblock back to channel-major
        pt = mp.tile([C, IH], F32, tag="mp", name=f"pt{d}{ih}")
        for isub in range(NSUB):
            nc.tensor.transpose(
                pt[:, JCH * isub : JCH * (isub + 1)],
                attT[:, C * isub : C * (isub + 1)],
                identity[:],
            )
        nc.vector.tensor_copy(fused_t[ih][C * d : C * (d + 1), :], pt[:])


def _build_program(
    ctx, tc, ct, mri, qsrc_ct, qsrc_mri, wqq, wv, woT, bo, ident, out
):
    nc = tc.nc
    wpool = ctx.enter_context(tc.tile_pool(name="wpool", bufs=1))
    featp = ctx.enter_context(tc.tile_pool(name="feat", bufs=2 * NFS))
    pools = {
        "qp": ctx.enter_context(tc.tile_pool(name="qp", bufs=2)),
        "vp": ctx.enter_context(tc.tile_pool(name="vp", bufs=2)),
        "ep": ctx.enter_context(tc.tile_pool(name="ep", bufs=4)),
        "np": ctx.enter_context(tc.tile_pool(name="npool", bufs=2)),
        "sp": ctx.enter_context(
            tc.tile_pool(name="spsum", bufs=2, space="PSUM")
        ),
        "pap": ctx.enter_context(
            tc.tile_pool(name="paccp", bufs=1, space="PSUM")
        ),
        "mp": ctx.enter_context(tc.tile_pool(name="mpsum", bufs=1, space="PSUM")),
    }
    fp = ctx.enter_context(tc.tile_pool(name="fusedp", bufs=2))
    op = ctx.enter_context(tc.tile_pool(name="outp", bufs=2))

    wqq_sb = wpool.tile([W, 2 * W], F16, name="wqq_sb")
    nc.sync.dma_start(wqq_sb[:], wqq[:])
    wv_sb = wpool.tile([W, 2 * W], F16, name="wv_sb")
    nc.sync.dma_start(wv_sb[:], wv[:])
    woT_sb = wpool.tile([2 * C, C], F32, name="woT_sb")
    nc.sync.dma_start(woT_sb[:], woT[:])
    bo_sb = wpool.tile([C, 1], F32, name="bo_sb")
    nc.sync.dma_start(bo_sb[:], bo[:])
    ident_sb = wpool.tile([JCH, JCH], F32, name="ident_sb")
    nc.sync.dma_start(ident_sb[:], ident[:])
    pools["identity"] = ident_sb

    fused_t = [
        fp.tile([2 * C, IH], F32, tag="fused", name=f"fused{ih}")
        for ih in range(NIH)
    ]

    # tiny query-source DMAs go first so they don't queue behind the 2 MB
    # of feature DMAs (HWDGE queues are FIFO); split in halves so the first
    # projection can start as soon as half arrives
    qsc = pools["qp"].tile([W, NQ], F16, tag="qsrc", name="qsc")
    qsm = pools["qp"].tile([W, NQ], F16, tag="qsrc", name="qsm")
    for h in range(NIH):
        nc.sync.dma_start(
            qsc[:, IH * h : IH * (h + 1)], qsrc_ct[:, IH * h : IH * (h + 1)]
        )
        nc.sync.dma_start(
            qsm[:, IH * h : IH * (h + 1)], qsrc_mri[:, IH * h : IH * (h + 1)]
        )

    # mri side first: it feeds direction 0 (CT queries over MRI K/V)
    fs_mri = _emit_feat_load(nc, featp, mri, "fsm", "fmri")
    fs_ct = _emit_feat_load(nc, featp, ct, "fsc", "fct")
    qq_d0 = _emit_qq_proj(nc, pools, wqq_sb, qsc, 0 * W, "qq_d0")
    vT_mri = _emit_v_proj(nc, pools, wv_sb, fs_mri, 0 * W, "vT_mri")

    # direction 0 while CT projections stream in behind it
    _emit_attention(nc, pools, fs_mri, qq_d0, vT_mri, fused_t, 0)

    qq_d1 = _emit_qq_proj(nc, pools, wqq_sb, qsm, 1 * W, "qq_d1")
    vT_ct = _emit_v_proj(nc, pools, wv_sb, fs_ct, 1 * W, "vT_ct")

    _emit_attention(nc, pools, fs_ct, qq_d1, vT_ct, fused_t, 1)

    for ih in range(NIH):
        po = pools["mp"].tile([C, IH], F32, tag="mp", name=f"po{ih}")
        nc.tensor.matmul(
            po[:], lhsT=woT_sb[:], rhs=fused_t[ih][:], start=True, stop=True
        )
        ot = op.tile([C, IH], F32, tag="ot", name=f"ot{ih}")
        nc.vector.tensor_scalar_add(ot[:], po[:], bo_sb[:])
        nc.sync.dma_start(out[:, IH * ih : IH * (ih + 1)], ot[:])


def build_bass():
    nc = bacc.Bacc("TRN2", target_bir_lowering=False, debug=False)
    ct = nc.dram_tensor("ct_feat", [W, N], F16, kind="ExternalInput").ap()
    mri = nc.dram_tensor("mri_feat", [W, N], F16, kind="ExternalInput").ap()
    qsrc_ct = nc.dram_tensor("qsrc_ct", [W, NQ], F16, kind="ExternalInput").ap()
    qsrc_mri = nc.dram_tensor("qsrc_mri", [W, NQ], F16, kind="ExternalInput").ap()
    wqq = nc.dram_tensor("wqq", [W, 2 * W], F16, kind="ExternalInput").ap()
    wv = nc.dram_tensor("wv", [W, 2 * W], F16, kind="ExternalInput").ap()
    woT = nc.dram_tensor("woT", [2 * C, C], F32, kind="ExternalInput").ap()
    bo = nc.dram_tensor("bo", [C, 1], F32, kind="ExternalInput").ap()
    ident = nc.dram_tensor("ident", [JCH, JCH], F32, kind="ExternalInput").ap()
    out = nc.dram_tensor("out", [C, NQ], F32, kind="ExternalOutput").ap()

    with tile.TileContext(nc) as tc, ExitStack() as ctx:
        _build_program(
            ctx, tc, ct, mri, qsrc_ct, qsrc_mri, wqq, wv, woT, bo, ident, out
        )
    nc.compile()
    return nc


def _aug(w, b):
    # (out,in) weight + (out,) bias -> lhsT-ready [w.T; b] of shape (in+1, out)
    return np.concatenate(
        [np.asarray(w, np.float32).T, np.asarray(b, np.float32)[None, :]], axis=0
    )


def _wv_pack(w, b):
    # (65, 65): [[wv.T; bv] | e_last]: extra column accumulates the denominator
    m = np.zeros((W, W), np.float32)
    m[:, :C] = _aug(w, b)
    m[C, C] = 1.0
    return m


def _wkb_pack(w, b):
    # (64, 65): [wk | bk] -- K projection folded into the query side
    return np.concatenate(
        [np.asarray(w, np.float32), np.asarray(b, np.float32)[:, None]], axis=1
    )


def prepare_inputs(inputs):
    scale = np.float32(1.0 / np.sqrt(C))
    ct = np.asarray(inputs["ct_features"], np.float32).reshape(C, N)
    mri = np.asarray(inputs["mri_features"], np.float32).reshape(C, N)
    ones = np.ones((1, N), np.float32)
    ct_aug = np.concatenate([ct, ones], axis=0).astype(np.float16)
    mri_aug = np.concatenate([mri, ones], axis=0).astype(np.float16)
    wq_ct = _aug(np.asarray(inputs["wq_ct"]) * scale, np.asarray(inputs["bq_ct"]) * scale)
    wq_mri = _aug(np.asarray(inputs["wq_mri"]) * scale, np.asarray(inputs["bq_mri"]) * scale)
    # compose Q projection with the query-side-folded K projection (fp32 host
    # matmul, rounded to fp16 once): q''_d = (Wq_aug @ [Wk|bk])^T @ qsrc_aug
    wqq = np.concatenate(
        [wq_ct @ _wkb_pack(inputs["wk_mri"], inputs["bk_mri"]),
         wq_mri @ _wkb_pack(inputs["wk_ct"], inputs["bk_ct"])],
        axis=1,
    ).astype(np.float16)
    wv = np.concatenate(
        [_wv_pack(inputs["wv_mri"], inputs["bv_mri"]),
         _wv_pack(inputs["wv_ct"], inputs["bv_ct"])],
        axis=1,
    ).astype(np.float16)
    woT = np.ascontiguousarray(np.asarray(inputs["wo"], np.float32).T)
    bo = np.ascontiguousarray(np.asarray(inputs["bo"], np.float32)[:, None])
    ident = np.eye(JCH, dtype=np.float32)

    in_maps = []
    for i in range(NCORES):
        sl = slice(NQ * i, NQ * (i + 1))
        in_maps.append(
            {
                "ct_feat": ct_aug,
                "mri_feat": mri_aug,
                "qsrc_ct": np.ascontiguousarray(ct_aug[:, sl]),
                "qsrc_mri": np.ascontiguousarray(mri_aug[:, sl]),
                "wqq": wqq,
                "wv": wv,
                "woT": woT,
                "bo": bo,
                "ident": ident,
            }
        )
    return in_maps


def assemble_output(results):
    out = np.concatenate([results[i]["out"] for i in range(NCORES)], axis=1)
    return out.reshape(1, C, 8, 32, 32)


_NC_CACHE = None


def _get_nc():
    global _NC_CACHE
    if _NC_CACHE is None:
        _NC_CACHE = build_bass()
    return _NC_CACHE


def kernel(**inputs):
    nc = _get_nc()
    in_maps = prepare_inputs(inputs)
    res = run_bass_kernel_spmd(nc, in_maps, list(range(NCORES)))
    return assemble_output(res.results)


if __name__ == "__main__":
    nc = build_bass()
    print("built OK")


# revision 35
# speedup vs baseline: 1.0893x; 1.0352x over previous
"""Trainium2 Bass kernel for CrossModalAttention.

Reference computation (B=1, C=64, N=8192 voxels):
  two cross-attention directions (CT queries over MRI keys/values and vice
  versa), each with an 8192x8192 attention matrix, fused output projection.

Sharding: each of the 8 cores owns 1024 query voxels for BOTH directions,
computes K/V over the full sequence locally (features are only 2 MB per
modality), and produces its own (64, 1024) slice of the output through the
final projection. No collectives; the host concatenates the 8 slices.

Device algorithm ("transposed world", no transposes of large tensors):
  The K projection is folded into the query side (associativity:
  (Wk f)^T q = f^T (Wk^T q)), so scores read the fp16 features directly:
    scores^T (j,i) = matmul(lhsT=feat_aug[:, j-chunk] (65,128),
                            rhs=q''_d (65,512)),  q''_d = [Wk^T q_d; bk.q_d]
  exp on ScalarE straight out of PSUM (max-subtraction skipped: |s| <= ~1.2),
  batched 3 PSUM banks per ACTIVATE to amortize the 352-cycle overhead.
  AV is flipped to out=(i,c) so the PE output partition dim is the full 128:
    att^T[i, 0:65] += matmul(lhsT=exp chunk (j,128i), rhs=V^T_aug (j,65))
  V^T_aug = feat_aug^T @ Wv' where Wv' carries the bias row and a final
  [0..0,1] column, so column 64 of att^T accumulates the softmax
  denominator for free. All four 128-query chains of one 512-query block
  share a single PSUM bank: only the first matmul uses start=True (clears
  the bank); later chains' first writes land on has_written=0 cells and
  overwrite, which initializes them correctly.
  Normalize = per-partition reciprocal of column 64 + tensor_scalar mult,
  then a 128x64 PE transpose per subblock rebuilds the channel-major
  fused tile for the fp32 final projection.

Precision: matmul operands are fp16 (PE streams 1 col/cycle with fast
weight load; fp32 is 4x slower, float32r 2x, and bf16's 8-bit mantissa
loses 10x accuracy for identical speed -- all values here are far inside
fp16 range). Accumulation is always fp32 in PSUM; softmax denominator,
normalization, transposes and the final projection are fp32. Rounding
errors of q/k/exp/v average out over the 8192-key softmax: end-to-end
error ~2e-4.
"""

from contextlib import ExitStack

import numpy as np

import concourse.bass as bass
import concourse.mybir as mybir
import concourse.tile as tile
from concourse import bacc
from concourse.bass_utils import run_bass_kernel_spmd

F32 = mybir.dt.float32
F16 = mybir.dt.float16
C = 64          # channels
N = 8192        # voxels (8*32*32)
NCORES = 8
NQ = N // NCORES      # 1024 queries per core
IH = 512              # query block (PSUM bank width in f32)
NIH = NQ // IH        # 2
NSUB = IH // 128      # 4 query subblocks per block (AV lhsT width)
JCH = 128             # key chunk (AV contraction tile)
NJ = N // JCH         # 64
GRP = 2               # score banks per exp group (2 banks -> one wide ACT op;
                      # 3 slots of 2 banks let AV lag a full group behind
                      # scores so the PE stream never waits on ScalarE)
VGW = 4               # vT chunks per projection group (4*65 f32 fits one bank)
W = C + 1             # 65: augmented channel dim
VCOLS = NJ * W        # vT storage: 64 chunks x 65 cols (65th col = denom ones)
NFS = 8               # feature DMA subtiles
FSW = N // NFS        # 1024 cols per subtile
JPS = FSW // JCH      # 8 j-chunks per feature subtile


def _emit_feat_load(nc, featp, feat_dram, tag, name):
    subs = []
    for s in range(NFS):
        t = featp.tile([W, FSW], F16, tag=tag, name=f"{name}{s}")
        nc.sync.dma_start(t[:], feat_dram[:, FSW * s : FSW * (s + 1)])
        subs.append(t)
    return subs


def _emit_qq_proj(nc, pools, wqq_sb, qsrc, wcol, name):
    """q''_d (65, NQ) = (Wq_aug @ [Wk|bk])^T @ qsrc_aug -- the Q projection and
    the K projection (folded onto the query side) composed on the host."""
    qp, sp = pools["qp"], pools["sp"]
    qq = qp.tile([W, NQ], F16, tag="qq", name=name)
    for h in range(NIH):
        pq = sp.tile([W, IH], F32, tag="ps", name=f"pqq_{name}{h}")
        nc.tensor.matmul(
            pq[:],
            lhsT=wqq_sb[:, wcol : wcol + W],
            rhs=qsrc[:, IH * h : IH * (h + 1)],
            start=True,
            stop=True,
        )
        nc.vector.tensor_copy(qq[:, IH * h : IH * (h + 1)], pq[:])
    return qq


def _emit_v_proj(nc, pools, wv_sb, fs, wcol, name):
    """vT_aug (128j x 65) chunks = feat_aug^T @ Wv' (ones column built in)."""
    vp, sp = pools["vp"], pools["sp"]
    vT = vp.tile([JCH, VCOLS], F16, tag="v", name=name)
    for g in range(NJ // VGW):
        pv = sp.tile([JCH, VGW * W], F32, tag="ps", name=f"pv_{name}{g}")
        for cc in range(VGW):
            j = VGW * g + cc
            nc.tensor.matmul(
                pv[:, W * cc : W * (cc + 1)],
                lhsT=fs[j // JPS][:, JCH * (j % JPS) : JCH * (j % JPS + 1)],
                rhs=wv_sb[:, wcol : wcol + W],
                start=True,
                stop=True,
            )
        nc.vector.tensor_copy(vT[:, W * VGW * g : W * VGW * (g + 1)], pv[:])
    return vT


def _emit_attention(nc, pools, fa, qq, vT, fused_t, d):
    sp, pap, mp, ep, npl = (
        pools["sp"], pools["pap"], pools["mp"], pools["ep"], pools["np"],
    )
    identity = pools["identity"]
    def emit_av(pacc, js, et):
        for idx, j in enumerate(js):
            for isub in range(NSUB):
                nc.tensor.matmul(
                    pacc[:, W * isub : W * (isub + 1)],
                    lhsT=et[:, IH * idx + JCH * isub : IH * idx + JCH * (isub + 1)],
                    rhs=vT[:, W * j : W * (j + 1)],
                    start=(j == 0 and isub == 0),
                    stop=(j == NJ - 1 and isub == NSUB - 1),
                    skip_group_check=True,
                )

    for ih in range(NIH):
        # one bank holds all four (128, 65) accumulation chains
        pacc = pap.tile([JCH, NSUB * W], F32, tag="pacc", name=f"pacc{d}{ih}")
        lag = None  # AV runs one score-group behind: exp is always ready
        for jg in range((NJ + GRP - 1) // GRP):
            js = list(range(GRP * jg, min(GRP * (jg + 1), NJ)))
            ps = sp.tile([JCH, GRP * IH], F32, tag="ps", name=f"ps{d}{ih}{jg}")
            for idx, j in enumerate(js):
                nc.tensor.matmul(
                    ps[:, IH * idx : IH * (idx + 1)],
                    lhsT=fa[j // JPS][:, JCH * (j % JPS) : JCH * (j % JPS + 1)],
                    rhs=qq[:, IH * ih : IH * (ih + 1)],
                    start=True,
                    stop=True,
                )
            et = ep.tile([JCH, GRP * IH], F16, tag="exp", name=f"et{d}{ih}{jg}")
            nc.scalar.activation(
                et[:, : IH * len(js)],
                ps[:, : IH * len(js)],
                mybir.ActivationFunctionType.Exp,
            )
            if lag is not None:
                emit_av(*lag)
            lag = (pacc, js, et)
        emit_av(*lag)
        # normalize per query (partition): r = 1 / denom-column
        r4 = npl.tile([JCH, NSUB], F32, tag="r4", name=f"r4{d}{ih}")
        nc.vector.reciprocal(
            r4[:].rearrange("p (i w) -> p i w", w=1),
            pacc[:].rearrange("p (i w) -> p i w", w=W)[:, :, C : C + 1],
        )
        attT = npl.tile([JCH, NSUB * C], F32, tag="attT", name=f"attT{d}{ih}")
        for isub in range(NSUB):
            nc.vector.tensor_scalar_mul(
                attT[:, C * isub : C * (isub + 1)],
                pacc[:, W * isub : W * isub + C],
                r4[:, isub : isub + 1],
            )
        # transpose each (128, 64) subblock back to channel-major
        pt = mp.tile([C, IH], F32, tag="mp", name=f"pt{d}{ih}")
        for isub in range(NSUB):
            nc.tensor.transpose(
                pt[:, JCH * isub : JCH * (isub + 1)],
                attT[:, C * isub : C * (isub + 1)],
                identity[:],
            )
        nc.vector.tensor_copy(fused_t[ih][C * d : C * (d + 1), :], pt[:])


def _build_program(
    ctx, tc, ct, mri, qsrc_ct, qsrc_mri, wqq, wv, woT, bo, ident, out
):
    nc = tc.nc
    wpool = ctx.enter_context(tc.tile_pool(name="wpool", bufs=1))
    featp = ctx.enter_context(tc.tile_pool(name="feat", bufs=2 * NFS))
    pools = {
        "qp": ctx.enter_context(tc.tile_pool(name="qp", bufs=2)),
        "vp": ctx.enter_context(tc.tile_pool(name="vp", bufs=2)),
        "ep": ctx.enter_context(tc.tile_pool(name="ep", bufs=4)),
        "np": ctx.enter_context(tc.tile_pool(name="npool", bufs=2)),
        "sp": ctx.enter_context(
            tc.tile_pool(name="spsum", bufs=3, space="PSUM")
        ),
        "pap": ctx.enter_context(
            tc.tile_pool(name="paccp", bufs=1, space="PSUM")
        ),
        "mp": ctx.enter_context(tc.tile_pool(name="mpsum", bufs=1, space="PSUM")),
    }
    fp = ctx.enter_context(tc.tile_pool(name="fusedp", bufs=2))
    op = ctx.enter_context(tc.tile_pool(name="outp", bufs=2))

    wqq_sb = wpool.tile([W, 2 * W], F16, name="wqq_sb")
    nc.sync.dma_start(wqq_sb[:], wqq[:])
    wv_sb = wpool.tile([W, 2 * W], F16, name="wv_sb")
    nc.sync.dma_start(wv_sb[:], wv[:])
    woT_sb = wpool.tile([2 * C, C], F32, name="woT_sb")
    nc.sync.dma_start(woT_sb[:], woT[:])
    bo_sb = wpool.tile([C, 1], F32, name="bo_sb")
    nc.sync.dma_start(bo_sb[:], bo[:])
    ident_sb = wpool.tile([JCH, JCH], F32, name="ident_sb")
    nc.sync.dma_start(ident_sb[:], ident[:])
    pools["identity"] = ident_sb

    fused_t = [
        fp.tile([2 * C, IH], F32, tag="fused", name=f"fused{ih}")
        for ih in range(NIH)
    ]

    # tiny query-source DMAs go first so they don't queue behind the 2 MB
    # of feature DMAs (HWDGE queues are FIFO); split in halves so the first
    # projection can start as soon as half arrives
    qsc = pools["qp"].tile([W, NQ], F16, tag="qsrc", name="qsc")
    qsm = pools["qp"].tile([W, NQ], F16, tag="qsrc", name="qsm")
    for h in range(NIH):
        nc.sync.dma_start(
            qsc[:, IH * h : IH * (h + 1)], qsrc_ct[:, IH * h : IH * (h + 1)]
        )
        nc.sync.dma_start(
            qsm[:, IH * h : IH * (h + 1)], qsrc_mri[:, IH * h : IH * (h + 1)]
        )

    # mri side first: it feeds direction 0 (CT queries over MRI K/V)
    fs_mri = _emit_feat_load(nc, featp, mri, "fsm", "fmri")
    fs_ct = _emit_feat_load(nc, featp, ct, "fsc", "fct")
    qq_d0 = _emit_qq_proj(nc, pools, wqq_sb, qsc, 0 * W, "qq_d0")
    vT_mri = _emit_v_proj(nc, pools, wv_sb, fs_mri, 0 * W, "vT_mri")

    # direction 0 while CT projections stream in behind it
    _emit_attention(nc, pools, fs_mri, qq_d0, vT_mri, fused_t, 0)

    qq_d1 = _emit_qq_proj(nc, pools, wqq_sb, qsm, 1 * W, "qq_d1")
    vT_ct = _emit_v_proj(nc, pools, wv_sb, fs_ct, 1 * W, "vT_ct")

    _emit_attention(nc, pools, fs_ct, qq_d1, vT_ct, fused_t, 1)

    for ih in range(NIH):
        po = pools["mp"].tile([C, IH], F32, tag="mp", name=f"po{ih}")
        nc.tensor.matmul(
            po[:], lhsT=woT_sb[:], rhs=fused_t[ih][:], start=True, stop=True
        )
        ot = op.tile([C, IH], F32, tag="ot", name=f"ot{ih}")
        nc.vector.tensor_scalar_add(ot[:], po[:], bo_sb[:])
        nc.sync.dma_start(out[:, IH * ih : IH * (ih + 1)], ot[:])


def build_bass():
    nc = bacc.Bacc("TRN2", target_bir_lowering=False, debug=False)
    ct = nc.dram_tensor("ct_feat", [W, N], F16, kind="ExternalInput").ap()
    mri = nc.dram_tensor("mri_feat", [W, N], F16, kind="ExternalInput").ap()
    qsrc_ct = nc.dram_tensor("qsrc_ct", [W, NQ], F16, kind="ExternalInput").ap()
    qsrc_mri = nc.dram_tensor("qsrc_mri", [W, NQ], F16, kind="ExternalInput").ap()
    wqq = nc.dram_tensor("wqq", [W, 2 * W], F16, kind="ExternalInput").ap()
    wv = nc.dram_tensor("wv", [W, 2 * W], F16, kind="ExternalInput").ap()
    woT = nc.dram_tensor("woT", [2 * C, C], F32, kind="ExternalInput").ap()
    bo = nc.dram_tensor("bo", [C, 1], F32, kind="ExternalInput").ap()
    ident = nc.dram_tensor("ident", [JCH, JCH], F32, kind="ExternalInput").ap()
    out = nc.dram_tensor("out", [C, NQ], F32, kind="ExternalOutput").ap()

    with tile.TileContext(nc) as tc, ExitStack() as ctx:
        _build_program(
            ctx, tc, ct, mri, qsrc_ct, qsrc_mri, wqq, wv, woT, bo, ident, out
        )
    nc.compile()
    return nc


def _aug(w, b):
    # (out,in) weight + (out,) bias -> lhsT-ready [w.T; b] of shape (in+1, out)
    return np.concatenate(
        [np.asarray(w, np.float32).T, np.asarray(b, np.float32)[None, :]], axis=0
    )


def _wv_pack(w, b):
    # (65, 65): [[wv.T; bv] | e_last]: extra column accumulates the denominator
    m = np.zeros((W, W), np.float32)
    m[:, :C] = _aug(w, b)
    m[C, C] = 1.0
    return m


def _wkb_pack(w, b):
    # (64, 65): [wk | bk] -- K projection folded into the query side
    return np.concatenate(
        [np.asarray(w, np.float32), np.asarray(b, np.float32)[:, None]], axis=1
    )


def prepare_inputs(inputs):
    scale = np.float32(1.0 / np.sqrt(C))
    ct = np.asarray(inputs["ct_features"], np.float32).reshape(C, N)
    mri = np.asarray(inputs["mri_features"], np.float32).reshape(C, N)
    ones = np.ones((1, N), np.float32)
    ct_aug = np.concatenate([ct, ones], axis=0).astype(np.float16)
    mri_aug = np.concatenate([mri, ones], axis=0).astype(np.float16)
    wq_ct = _aug(np.asarray(inputs["wq_ct"]) * scale, np.asarray(inputs["bq_ct"]) * scale)
    wq_mri = _aug(np.asarray(inputs["wq_mri"]) * scale, np.asarray(inputs["bq_mri"]) * scale)
    # compose Q projection with the query-side-folded K projection (fp32 host
    # matmul, rounded to fp16 once): q''_d = (Wq_aug @ [Wk|bk])^T @ qsrc_aug
    wqq = np.concatenate(
        [wq_ct @ _wkb_pack(inputs["wk_mri"], inputs["bk_mri"]),
         wq_mri @ _wkb_pack(inputs["wk_ct"], inputs["bk_ct"])],
        axis=1,
    ).astype(np.float16)
    wv = np.concatenate(
        [_wv_pack(inputs["wv_mri"], inputs["bv_mri"]),
         _wv_pack(inputs["wv_ct"], inputs["bv_ct"])],
        axis=1,
    ).astype(np.float16)
    woT = np.ascontiguousarray(np.asarray(inputs["wo"], np.float32).T)
    bo = np.ascontiguousarray(np.asarray(inputs["bo"], np.float32)[:, None])
    ident = np.eye(JCH, dtype=np.float32)

    in_maps = []
    for i in range(NCORES):
        sl = slice(NQ * i, NQ * (i + 1))
        in_maps.append(
            {
                "ct_feat": ct_aug,
                "mri_feat": mri_aug,
                "qsrc_ct": np.ascontiguousarray(ct_aug[:, sl]),
                "qsrc_mri": np.ascontiguousarray(mri_aug[:, sl]),
                "wqq": wqq,
                "wv": wv,
                "woT": woT,
                "bo": bo,
                "ident": ident,
            }
        )
    return in_maps


def assemble_output(results):
    out = np.concatenate([results[i]["out"] for i in range(NCORES)], axis=1)
    return out.reshape(1, C, 8, 32, 32)


_NC_CACHE = None


def _get_nc():
    global _NC_CACHE
    if _NC_CACHE is None:
        _NC_CACHE = build_bass()
    return _NC_CACHE


def kernel(**inputs):
    nc = _get_nc()
    in_maps = prepare_inputs(inputs)
    res = run_bass_kernel_spmd(nc, in_maps, list(range(NCORES)))
    return assemble_output(res.results)


if __name__ == "__main__":
    nc = build_bass()
    print("built OK")
